# revision 12
# baseline (speedup 1.0000x reference)
"""BCQLinear packed forward on 8 Trainium2 NeuronCores.

Column-parallel sharding: binary/alpha/bias sharded along out_features
(4096 -> 8 x 512); input activations replicated. Per core:

  1. Dequant on DVE via per-partition-scalar ops (scalar_tensor_tensor):
     W[o, g, a] = sum_b alpha[o, g, b] * B[o, g, a, b], accumulated in bf16.
  2. Transpose W -> Wt[a, g, o] with the XBAR DMA-transpose (no PE work).
  3. bf16 matmuls, g-major waves: out[t, o] += x[t, g*128+a] * Wt[a, g, o]
     with 8 PSUM accumulators (one per 128-token block of the active half).
  4. Bias add on DVE, f32 store.

x is host-staged transposed ([i, tokens]) in bf16 so the contraction dim
lands on partitions with >=512B contiguous DMA runs. Weight-path DMAs
issue on SP, x/out DMAs on ACT so the two streams don't head-of-line
block each other.

Shapes hardcoded for this instance:
  input  [2, 1024, 4096] f32 -> out [2, 1024, 4096] f32
  binary [4096, 32, 128, 3] (+-1), alpha [4096, 32, 3], bias [4096]
"""

import numpy as np
from contextlib import ExitStack

import ml_dtypes
import bass_rust
import concourse.bass as bass
import concourse.mybir as mybir
import concourse.tile as tile
from concourse.bass_utils import run_bass_kernel_spmd


def _legalize_waits(nc, max_waits=1):
    """Walrus codegen allows only one sync-wait on (at least) DVE
    TensorTensor instructions. Move excess waits onto injected same-engine
    NoOps placed immediately before the instruction (program order per
    engine preserves the semantics)."""
    seq = 0
    for fn in nc.m.functions:
        for blk in fn.blocks:
            new_insts = []
            changed = False
            for inst in blk.instructions:
                si = inst.sync_info
                if si is not None and len(si.on_wait) > max_waits:
                    waits = list(si.on_wait)
                    for w in waits[:-max_waits]:
                        nop = mybir.InstNoOp(name=f"wlegal-{seq}")
                        seq += 1
                        nop.engine = inst.engine
                        nop.sync_info = bass_rust.SyncInfo(
                            on_wait=[w], on_update=[])
                        new_insts.append(nop)
                    inst.sync_info = bass_rust.SyncInfo(
                        on_wait=waits[-max_waits:],
                        on_update=list(si.on_update))
                    changed = True
                new_insts.append(inst)
            if changed:
                blk.instructions = new_insts

P = 128          # partitions
N_CORES = 8
B, S = 2, 1024
MS = B * S       # 2048 tokens
I = 4096         # in_features
O = 4096         # out_features
O_SH = O // N_CORES  # 512 per core
G, A, NB = 32, 128, 3
KT = I // P      # 32 contraction tiles (== G since A == P)
MB = MS // P     # 16 token blocks
OT = O_SH // P   # 4 o-tiles per core

F32 = mybir.dt.float32
BF16 = mybir.dt.bfloat16
FP8 = mybir.dt.float8e4

_CACHED = {}

mult = mybir.AluOpType.mult
add = mybir.AluOpType.add

XCK = 4          # m-blocks (128 tokens each) per x chunk
NCH = MB // XCK  # 4 chunks
XQ = 4           # k-quarter DMAs per chunk
KQ = KT // XQ


def build_nc(gh_sz: int = 8, b_bufs: int = 12, w_bufs: int = 8,
             o_bufs: int = 2) -> bass.Bass:
    GH = G // gh_sz  # number of g-chunks

    nc = bass.Bass("TRN2", target_bir_lowering=False, debug=False)

    # Host-staged layouts (pure relayouts/casts of the sharded inputs):
    #  xt    [KT, P, MS] bf16 : xt[k, p, t] = x[t, k*128+p]
    #  bperm [NB, O_SH, G, A] fp8 : bit-plane-major binary shard
    #  al    [P, OT*G*NB] f32 : al[p, ot*G*NB + g*NB + b] = alpha[ot*128+p, g, b]
    #  biasr [P, O_SH] f32 : bias shard replicated across partitions
    xt_d = nc.dram_tensor("xt", [KT, P, MS], BF16, kind="ExternalInput").ap()
    b_d = nc.dram_tensor("bperm", [NB, O_SH, G, A], FP8, kind="ExternalInput").ap()
    al_d = nc.dram_tensor("al", [P, OT * G * NB], F32, kind="ExternalInput").ap()
    bias_d = nc.dram_tensor("biasr", [P, O_SH], F32, kind="ExternalInput").ap()
    out_d = nc.dram_tensor("out", [MS, O_SH], F32, kind="ExternalOutput").ap()
    out_t = out_d.rearrange("(mb p) o -> mb p o", p=P)
    xt_p = xt_d.rearrange("k p t -> p k t")

    with tile.TileContext(nc) as tc, ExitStack() as ctx:
        const = ctx.enter_context(tc.tile_pool(name="const", bufs=1))
        xpool = ctx.enter_context(tc.tile_pool(name="x", bufs=1))
        bpool = ctx.enter_context(tc.tile_pool(name="bin", bufs=b_bufs))
        wpool = ctx.enter_context(tc.tile_pool(name="w", bufs=w_bufs))
        wtpool = ctx.enter_context(tc.tile_pool(name="wt", bufs=1))
        opool = ctx.enter_context(tc.tile_pool(name="o", bufs=o_bufs))
        ps = ctx.enter_context(tc.tile_pool(name="ps", bufs=1, space="PSUM"))

        al_sb = const.tile([P, OT * G * NB], F32)
        nc.sync.dma_start(al_sb, al_d)
        al4 = al_sb.rearrange("p (ot g nb) -> p ot g nb", ot=OT, nb=NB)
        bias_sb = const.tile([P, O_SH], F32)
        nc.sync.dma_start(bias_sb, bias_d)

        # Wt[a, g, o] resident for the whole run (both halves).
        wt = wtpool.tile([P, G, O_SH], BF16)

        # x: 16 k-quarter tiles [P, KQ, XCK*128tok], all resident; loaded on
        # ACT so x streams independently of the weight-path DMAs on SP.
        x_tiles = [[None] * XQ for _ in range(NCH)]

        def load_x(c, q):
            t = xpool.tile([P, KQ, XCK * P], BF16, tag=f"x{c}q{q}")
            x_tiles[c][q] = t
            tsl = slice(c * XCK * P, (c + 1) * XCK * P)
            ksl = slice(q * KQ, (q + 1) * KQ)
            nc.scalar.dma_start(t, xt_p[:, ksl, tsl])

        def emit_b_dmas(ot, gh):
            gsl = slice(gh * gh_sz, (gh + 1) * gh_sz)
            b_tiles = []
            for b in range(NB):
                bt = bpool.tile([P, gh_sz * A], FP8)
                nc.sync.dma_start(
                    bt,
                    b_d[b, ot * P:(ot + 1) * P, gsl].rearrange(
                        "p g a -> p (g a)"))
                b_tiles.append(bt)
            return b_tiles

        def dequant(ot, gh, b_tiles):
            """Dequant g-chunk gh of o-tile ot into a staging tile, then
            DMA-transpose it into Wt[:, gh-slice, ot-slice]."""
            w = wpool.tile([P, gh_sz * A], BF16)
            for go in range(gh_sz):
                g = gh * gh_sz + go
                wsl = w[:, go * A:(go + 1) * A]
                bsl = [bt[:, go * A:(go + 1) * A] for bt in b_tiles]
                nc.vector.tensor_scalar(
                    wsl, bsl[0], al4[:, ot, g, 0:1], None, op0=mult)
                nc.vector.scalar_tensor_tensor(
                    wsl, bsl[1], al4[:, ot, g, 1:2], wsl, op0=mult, op1=add)
                nc.vector.scalar_tensor_tensor(
                    wsl, bsl[2], al4[:, ot, g, 2:3], wsl, op0=mult, op1=add)
            gsl = slice(gh * gh_sz, (gh + 1) * gh_sz)
            nc.sync.dma_start_transpose(wt[:, gsl, ot * P:(ot + 1) * P], w)

        ps_tiles = [None] * MB

        def mm_wave(half, g):
            """One g-wave of matmuls for all 8 m-blocks of the half."""
            q, kq = g // KQ, g % KQ
            for mi in range(8):
                m = half * 8 + mi
                c, ts = m // XCK, (m % XCK) * P
                if g == 0:
                    ps_tiles[m] = ps.tile([P, O_SH], F32, tag=f"ps{mi}",
                                          name=f"ps_m{m}")
                nc.tensor.matmul(
                    ps_tiles[m], x_tiles[c][q][:, kq, ts:ts + P],
                    wt[:, g], start=(g == 0), stop=(g == G - 1))

        def finish_m(m):
            out_sb = opool.tile([P, O_SH], F32)
            nc.vector.tensor_tensor(out_sb, ps_tiles[m], bias_sb, add)
            nc.scalar.dma_start(out_t[m], out_sb)

        # ---- Schedule ----
        load_x(0, 0)
        load_x(1, 0)

        # Half 0: dequant pipelined ahead of the matmul waves, g-major.
        for gh in range(GH):
            bts = [emit_b_dmas(ot, gh) for ot in range(OT)]
            for ot in range(OT):
                dequant(ot, gh, bts[ot])
            if gh == 0:
                load_x(0, 1)
                load_x(1, 1)
            for g in range(gh * gh_sz, (gh + 1) * gh_sz):
                mm_wave(0, g)
            if gh == 0:
                for cq in [(0, 2), (1, 2), (0, 3), (1, 3)]:
                    load_x(*cq)
            elif gh == 1:
                for q in range(XQ):
                    load_x(2, q)
            elif gh == 2:
                for q in range(XQ):
                    load_x(3, q)
        for m in range(8):
            finish_m(m)

        # Half 1: Wt resident, pure matmul throughput.
        for g in range(G):
            mm_wave(1, g)
        for m in range(8, 16):
            finish_m(m)

    _legalize_waits(nc)
    return nc


def _stage_inputs(input, binary, alpha, bias):
    x = np.ascontiguousarray(
        np.asarray(input, dtype=np.float32)).reshape(MS, I)
    # xt[k, p, t] = x[t, k*128+p]
    xt = np.ascontiguousarray(x.T.reshape(KT, P, MS)).astype(ml_dtypes.bfloat16)
    # +-1 binary is exactly representable in fp8e4: lossless cast.
    bperm = np.ascontiguousarray(
        np.asarray(binary, dtype=np.float32).transpose(3, 0, 1, 2)
    ).astype(ml_dtypes.float8_e4m3fn)
    alpha = np.ascontiguousarray(np.asarray(alpha, dtype=np.float32))
    bias = np.asarray(bias, dtype=np.float32)

    in_maps = []
    for c in range(N_CORES):
        sl = slice(c * O_SH, (c + 1) * O_SH)
        al = np.ascontiguousarray(
            alpha[sl].reshape(OT, P, G, NB).transpose(1, 0, 2, 3)
        ).reshape(P, OT * G * NB)
        in_maps.append({
            "xt": xt,
            "bperm": np.ascontiguousarray(bperm[:, sl]),
            "al": al,
            "biasr": np.ascontiguousarray(
                np.broadcast_to(bias[sl][None, :], (P, O_SH))),
        })
    return in_maps


def kernel(input, binary, alpha, bias, _trace=False, **_kw):
    key = ()
    if key not in _CACHED:
        _CACHED[key] = build_nc()
    nc = _CACHED[key]
    in_maps = _stage_inputs(input, binary, alpha, bias)
    res = run_bass_kernel_spmd(nc, in_maps, core_ids=list(range(N_CORES)),
                               trace=_trace)
    out = np.concatenate([res.results[c]["out"] for c in range(N_CORES)],
                         axis=1).reshape(B, S, O).astype(np.float32)
    if _trace:
        kernel.last_result = res
    return out
